# revision 1
# baseline (speedup 1.0000x reference)
import sys

sys.path.insert(0, "/opt/trn_rl_repo")

import numpy as np
import ml_dtypes

import concourse.bacc as bacc
import concourse.mybir as mybir
from concourse.tile import TileContext
from concourse import bass_utils

F32 = mybir.dt.float32
F32R = mybir.dt.float32r
BF16 = mybir.dt.bfloat16
AF = mybir.ActivationFunctionType
ALU = mybir.AluOpType

B, T, D = 8, 1024, 1024
H, E = 16, 64
DFF = 4 * D
NCORE = 8
NT = T // 128
ND = D // 128
NJ = DFF // 128
LN_EPS = 1e-5


class _Done(Exception):
    pass


def r32(ap):
    return ap.bitcast(F32R)


def v32(ap):
    return ap.bitcast(F32)


def _build(upto=9, reps=1):
    nc = bacc.Bacc("TRN2", target_bir_lowering=False, debug=False,
                   num_devices=NCORE)

    x_l = nc.dram_tensor("x_l", [T, D], F32R, kind="ExternalInput")
    wqp = nc.dram_tensor("wqp", [H // 2, 128, ND, 128], F32R,
                         kind="ExternalInput")
    wkp = nc.dram_tensor("wkp", [H // 2, 128, ND, 128], F32R,
                         kind="ExternalInput")
    wva = nc.dram_tensor("wva", [D, D], F32R, kind="ExternalInput")
    wpj = nc.dram_tensor("wpj", [D, D], F32R, kind="ExternalInput")
    w1r = nc.dram_tensor("w1r", [NJ, 128, ND, 128], F32R,
                         kind="ExternalInput")
    w2b = nc.dram_tensor("w2b", [DFF, D], BF16, kind="ExternalInput")
    g1f = nc.dram_tensor("g1f", [128, ND], F32, kind="ExternalInput")
    be1f = nc.dram_tensor("be1f", [128, ND], F32, kind="ExternalInput")
    g2f = nc.dram_tensor("g2f", [128, ND], F32, kind="ExternalInput")
    be2f = nc.dram_tensor("be2f", [128, ND], F32, kind="ExternalInput")
    bpf = nc.dram_tensor("bpf", [128, ND], F32, kind="ExternalInput")
    b1f = nc.dram_tensor("b1f", [128, NJ], F32, kind="ExternalInput")
    b2b = nc.dram_tensor("b2b", [128, D], F32, kind="ExternalInput")
    idn = nc.dram_tensor("idn", [128, 128], F32R, kind="ExternalInput")
    onz = nc.dram_tensor("onz", [128, 128], F32R, kind="ExternalInput")
    ond = nc.dram_tensor("ond", [128, 1], F32R, kind="ExternalInput")
    mby = nc.dram_tensor("mby", [128, 128], BF16, kind="ExternalInput")
    out_l = nc.dram_tensor("out_l", [T, D], F32, kind="ExternalOutput")

    def dump_fm(tiles, n):
        for c in range(n):
            nc.sync.dma_start(out_l[128 * c:128 * (c + 1), :],
                              v32(tiles[c][:]))

    with TileContext(nc) as tc:
        with (
            tc.tile_pool(name="fm", bufs=16) as fm,
            tc.tile_pool(name="const", bufs=1) as cp,
        ):
          try:
            def cload(name, dram, shape, dtype=F32):
                t = cp.tile(list(shape), dtype, tag=name, name=name)
                nc.sync.dma_start(t[:], dram[:])
                return t

            c_id = cload("idn", idn, [128, 128], F32R)
            c_idb = cp.tile([128, 128], BF16, tag="idnb", name="idnb")
            nc.vector.tensor_copy(c_idb[:], c_id[:])
            c_on = cload("onz", onz, [128, 128], F32R)
            c_od = cload("ond", ond, [128, 1], F32R)
            c_g1 = cload("g1f", g1f, [128, ND])
            c_be1 = cload("be1f", be1f, [128, ND])
            c_g2 = cload("g2f", g2f, [128, ND])
            c_be2 = cload("be2f", be2f, [128, ND])
            c_bp = cload("bpf", bpf, [128, ND])
            c_b1 = cload("b1f", b1f, [128, NJ])
            c_b2 = cload("b2b", b2b, [128, D])
            c_mb = cload("mby", mby, [128, 128], BF16)
            c_eps = cp.tile([128, 1], F32, tag="eps", name="eps")
            nc.gpsimd.memset(c_eps[:], LN_EPS)

            def _phases():
                xf = [fm.tile([128, T], F32R, tag="fm", name="xf")
                      for _ in range(ND)]
                qT, kT, v_sb = [], [], []
                with (
                    tc.tile_pool(name="qk", bufs=16) as qkp,
                    tc.tile_pool(name="vsb", bufs=8) as vp,
                ):
                  with (
                    tc.tile_pool(name="xin", bufs=3) as xp,
                    tc.tile_pool(name="scr", bufs=2) as scrp,
                    tc.tile_pool(name="st1", bufs=4) as st1,
                    tc.tile_pool(name="ps_a", bufs=1, space="PSUM") as psa,
                  ):
                    for m in range(NT):
                        xm = xp.tile([128, D], F32R, tag="xin")
                        nc.sync.dma_start(xm[:], x_l[128 * m:128 * (m + 1), :])
                        scr = scrp.tile([128, D], F32, tag="scr", name="scr")
                        st = st1.tile([128, 4], F32, tag="st", name="st")
                        nc.scalar.activation(scr[:], xm[:], AF.Square,
                                             accum_out=st[:, 1:2])
                        nc.scalar.activation(scr[:], xm[:], AF.Identity,
                                             accum_out=st[:, 0:1])
                        nc.vector.tensor_scalar_mul(st[:, 0:1], st[:, 0:1],
                                                    1.0 / D)
                        nc.vector.tensor_scalar_mul(st[:, 1:2], st[:, 1:2],
                                                    1.0 / D)
                        nc.vector.tensor_mul(st[:, 2:3], st[:, 0:1],
                                             st[:, 0:1])
                        nc.vector.tensor_sub(st[:, 2:3], st[:, 1:2],
                                             st[:, 2:3])
                        nc.scalar.activation(st[:, 2:3], st[:, 2:3], AF.Sqrt,
                                             bias=c_eps[:])
                        nc.vector.reciprocal(st[:, 3:4], st[:, 2:3])
                        nc.vector.tensor_scalar(
                            out=xm[:], in0=xm[:], scalar1=st[:, 0:1],
                            scalar2=st[:, 3:4], op0=ALU.subtract, op1=ALU.mult)
                        for c in range(ND):
                            pt = psa.tile([128, 128], F32R, tag="tr",
                                          bufs=3, name="pt")
                            nc.tensor.transpose(
                                pt[:], xm[:, 128 * c:128 * (c + 1)], c_id[:])
                            nc.vector.tensor_scalar(
                                out=xf[c][:, 128 * m:128 * (m + 1)],
                                in0=pt[:], scalar1=c_g1[:, c:c + 1],
                                scalar2=c_be1[:, c:c + 1],
                                op0=ALU.mult, op1=ALU.add)
                    if upto == 1:
                        dump_fm(xf, ND)
                        raise _Done()
                    xln1 = xf

                    with (
                        tc.tile_pool(name="wq", bufs=2) as wqpool,
                        tc.tile_pool(name="wk", bufs=2) as wkpool,
                    ):
                        for p in range(H // 2):
                            wq_t = wqpool.tile([128, D], F32R, tag="wq")
                            nc.sync.dma_start(
                                wq_t[:], wqp[p].rearrange("a b c -> a (b c)"))
                            wk_t = wkpool.tile([128, D], F32R, tag="wk")
                            nc.sync.dma_start(
                                wk_t[:], wkp[p].rearrange("a b c -> a (b c)"))
                            q_t = qkp.tile([128, T], BF16, tag="qk")
                            k_t = qkp.tile([128, T], BF16, tag="qk")
                            for dst, w_t in ((q_t, wq_t), (k_t, wk_t)):
                                for jj in range(2):
                                    sl = slice(512 * jj, 512 * (jj + 1))
                                    ps = psa.tile([128, 512], F32, tag="qk",
                                                  bufs=3, name="psqk")
                                    for c in range(ND):
                                        nc.tensor.matmul(
                                            ps[:],
                                            w_t[:, 128 * c:128 * (c + 1)],
                                            xln1[c][:, sl],
                                            start=(c == 0),
                                            stop=(c == ND - 1))
                                    nc.vector.tensor_copy(dst[:, sl], ps[:])
                            qT.append(q_t)
                            kT.append(k_t)
                    with tc.tile_pool(name="wv", bufs=8) as wvpool:
                        wv_t = []
                        for c in range(ND):
                            w = wvpool.tile([128, D], F32R, tag="wv",
                                            name="wv_t")
                            nc.sync.dma_start(
                                w[:], wva[128 * c:128 * (c + 1), :])
                            wv_t.append(w)
                        for i in range(NT):
                            vt = vp.tile([128, H * 65], BF16, tag="v",
                                         name="vt")
                            v3 = vt.rearrange("p (h e) -> p h e", e=65)
                            nc.gpsimd.memset(v3[:, :, 64:65], 1.0)
                            for nb in range(2):
                                ps = psa.tile([128, 512], F32, tag="v",
                                              bufs=2, name="psv")
                                for c in range(ND):
                                    nc.tensor.matmul(
                                        ps[:],
                                        xln1[c][:, 128 * i:128 * (i + 1)],
                                        wv_t[c][:, 512 * nb:512 * (nb + 1)],
                                        start=(c == 0), stop=(c == ND - 1))
                                nc.scalar.copy(
                                    v3[:, 8 * nb:8 * (nb + 1), 0:64],
                                    ps[:].rearrange("p (h e) -> p h e", e=64))
                            v_sb.append(vt)
                    if upto == 2:
                        for c in range(2):
                            nc.sync.dma_start(
                                out_l[128 * c:128 * (c + 1), :],
                                v32(qT[c][:]))
                            nc.sync.dma_start(
                                out_l[128 * (c + 2):128 * (c + 3), :],
                                v32(kT[c][:]))
                        raise _Done()
                  aoT = [fm.tile([128, T], F32R, tag="fm", name="aoT")
                         for _ in range(ND)]
                  with (
                      tc.tile_pool(name="atm", bufs=8) as atp,
                      tc.tile_pool(name="sc", bufs=12) as scp,
                      tc.tile_pool(name="rc", bufs=8) as rcp,
                      tc.tile_pool(name="wpj", bufs=8) as wpjp,
                      tc.tile_pool(name="ps_b", bufs=1, space="PSUM") as psb,
                  ):
                      at_t = [atp.tile([128, D], BF16, tag="atm",
                                       name="at_t") for _ in range(NT)]
                      for h in range(H):
                          p, q = h // 2, h % 2
                          qsl = slice(64 * q, 64 * (q + 1))
                          se = []
                          for i in range(NT):
                              st = scp.tile([128, T], BF16, tag="sc",
                                            name="se")
                              if i < 4:
                                  sp0 = psb.tile([128, 512], F32, tag="sc",
                                                 bufs=3, name="sp0")
                                  nc.tensor.matmul(
                                      sp0[:],
                                      kT[p][qsl, 128 * i:128 * (i + 1)],
                                      qT[p][qsl, 0:512],
                                      start=True, stop=True)
                                  nc.scalar.activation(
                                      st[:, 128 * i:512],
                                      sp0[:, 128 * i:512],
                                      AF.Exp, scale=0.125)
                              sp1 = psb.tile([128, 512], F32, tag="sc",
                                             bufs=3, name="sp1")
                              nc.tensor.matmul(
                                  sp1[:],
                                  kT[p][qsl, 128 * i:128 * (i + 1)],
                                  qT[p][qsl, 512:1024],
                                  start=True, stop=True)
                              lo = max(0, 128 * i - 512)
                              nc.scalar.activation(
                                  st[:, 512 + lo:], sp1[:, lo:],
                                  AF.Exp, scale=0.125)
                              dg = slice(128 * i, 128 * (i + 1))
                              nc.vector.tensor_mul(
                                  st[:, dg], st[:, dg], c_mb[:])
                              se.append(st)
                          for m in range(NT):
                              av = psb.tile([128, 65], F32, tag="av",
                                            bufs=2, name="av")
                              for i in range(m + 1):
                                  nc.tensor.matmul(
                                      av[:],
                                      se[i][:, 128 * m:128 * (m + 1)],
                                      v_sb[i].rearrange(
                                          "p (h e) -> p h e", e=65)[:, h, :],
                                      start=(i == 0), stop=(i == m))
                              rcol = rcp.tile([128, 1], F32, tag="rc",
                                              name="rc")
                              nc.vector.reciprocal_approx_fast(
                                  rcol[:], av[:, 64:65])
                              nc.vector.tensor_scalar_mul(
                                  at_t[m][:, 64 * h:64 * (h + 1)],
                                  av[:, 0:64], rcol[:])
                      for m in range(NT):
                          for c in range(ND):
                              pt = psb.tile([128, 128], BF16, tag="tr2",
                                            bufs=1, name="pt2")
                              nc.tensor.transpose(
                                  pt[:], at_t[m][:, 128 * c:128 * (c + 1)],
                                  c_idb[:])
                              nc.vector.tensor_copy(
                                  aoT[c][:, 128 * m:128 * (m + 1)], pt[:])
                      if upto == 3:
                          dump_fm(aoT, ND)
                          raise _Done()

                      wp_t = []
                      for c in range(ND):
                          w = wpjp.tile([128, D], F32R, tag="wpj")
                          nc.sync.dma_start(
                              w[:], wpj[128 * c:128 * (c + 1), :])
                          wp_t.append(w)
                      for co in range(ND):
                          for jj in range(2):
                              sl = slice(512 * jj, 512 * (jj + 1))
                              ps = psb.tile([128, 512], F32, tag="pj",
                                            bufs=2, name="pspj")
                              for c in range(ND):
                                  nc.tensor.matmul(
                                      ps[:],
                                      wp_t[c][:, 128 * co:128 * (co + 1)],
                                      aoT[c][:, sl],
                                      start=(c == 0), stop=(c == ND - 1))
                              nc.vector.scalar_tensor_tensor(
                                  out=xln1[co][:, sl], in0=ps[:],
                                  scalar=c_bp[:, co:co + 1],
                                  in1=xln1[co][:, sl],
                                  op0=ALU.add, op1=ALU.add)
                x2 = xln1
                if upto == 4:
                    dump_fm(x2, ND)
                    raise _Done()

                with (
                    tc.tile_pool(name="sq2", bufs=3) as sq2,
                    tc.tile_pool(name="rw2", bufs=1) as rw2,
                    tc.tile_pool(name="ps_c", bufs=1, space="PSUM") as psc,
                ):
                    sq = []
                    for c in range(ND):
                        s = sq2.tile([128, T], F32R, tag="sq", name="sq")
                        nc.vector.tensor_mul(s[:], x2[c][:], x2[c][:])
                        sq.append(s)
                    st_x = [psc.tile([1, 512], F32, tag="st", bufs=4,
                                     name="st_x") for _ in range(2)]
                    st_q = [psc.tile([1, 512], F32, tag="st", bufs=4,
                                     name="st_q") for _ in range(2)]
                    for jj in range(2):
                        sl = slice(512 * jj, 512 * (jj + 1))
                        for c in range(ND):
                            nc.tensor.matmul(
                                st_x[jj][:], c_od[:], x2[c][:, sl],
                                start=(c == 0), stop=(c == ND - 1))
                        for c in range(ND):
                            nc.tensor.matmul(
                                st_q[jj][:], c_od[:], sq[c][:, sl],
                                start=(c == 0), stop=(c == ND - 1))
                    mu_r = rw2.tile([1, T], F32, tag="mu_r")
                    ms_r = rw2.tile([1, T], F32, tag="ms_r")
                    sd_r = rw2.tile([1, T], F32, tag="sd_r")
                    rc_r = rw2.tile([1, T], F32, tag="rc_r")
                    for jj in range(2):
                        sl = slice(512 * jj, 512 * (jj + 1))
                        nc.vector.tensor_copy(mu_r[:, sl], st_x[jj][:])
                        nc.vector.tensor_copy(ms_r[:, sl], st_q[jj][:])
                    nc.vector.tensor_mul(sd_r[:], mu_r[:], mu_r[:])
                    nc.vector.tensor_sub(sd_r[:], ms_r[:], sd_r[:])
                    nc.scalar.activation(sd_r[:], sd_r[:], AF.Sqrt,
                                         bias=c_eps[0:1, :])
                    nc.vector.reciprocal_approx_fast(rc_r[:], sd_r[:])
                    mu_b = rw2.tile([128, T], F32R, tag="mu_b")
                    r_b = rw2.tile([128, T], F32R, tag="r_b")
                    for jj in range(2):
                        sl = slice(512 * jj, 512 * (jj + 1))
                        pm = psc.tile([128, 512], F32, tag="bc", bufs=2,
                                      name="pm")
                        nc.tensor.matmul(pm[:], v32(c_on[0:1, :]),
                                         mu_r[:, sl])
                        nc.vector.tensor_copy(mu_b[:, sl], pm[:])
                        pr = psc.tile([128, 512], F32, tag="bc", bufs=2,
                                      name="pr")
                        nc.tensor.matmul(pr[:], v32(c_on[0:1, :]),
                                         rc_r[:, sl])
                        nc.vector.tensor_copy(r_b[:, sl], pr[:])
                    for c in range(ND):
                        nc.vector.tensor_sub(x2[c][:], x2[c][:], mu_b[:])
                        nc.vector.tensor_mul(x2[c][:], x2[c][:], r_b[:])
                        nc.vector.tensor_scalar(
                            out=x2[c][:], in0=x2[c][:],
                            scalar1=c_g2[:, c:c + 1],
                            scalar2=c_be2[:, c:c + 1],
                            op0=ALU.mult, op1=ALU.add)
                    xln2 = x2
                    if upto == 5:
                        dump_fm(xln2, ND)
                        raise _Done()

                    xp2 = [fm.tile([128, D], F32, tag="fm", name="xp2")
                           for _ in range(NT)]
                    for m in range(NT):
                        for c in range(ND):
                            sl = slice(128 * c, 128 * (c + 1))
                            pt = psc.tile([128, 128], F32R, tag="tr3",
                                          bufs=2, name="pt3")
                            nc.tensor.transpose(
                                pt[:], xln2[c][:, 128 * m:128 * (m + 1)],
                                c_id[:])
                            nc.vector.tensor_add(xp2[m][:, sl], pt[:],
                                                 c_b2[:, sl])

                with (
                    tc.tile_pool(name="acc", bufs=8) as accp,
                    tc.tile_pool(name="hj", bufs=16) as hjp,
                    tc.tile_pool(name="w1t", bufs=3) as w1p,
                    tc.tile_pool(name="w2t", bufs=16) as w2p,
                    tc.tile_pool(name="ps_d", bufs=1, space="PSUM") as psd,
                ):
                    acc = [accp.tile([128, D], F32, tag="acc", name="acc")
                           for _ in range(NT)]
                    hjs = [[hjp.tile([128, T], BF16, tag="hj", name="hj")
                            for _ in range(8)] for _ in range(2)]
                    w2s = [[w2p.tile([128, D], BF16, tag="w2", name="w2t")
                            for _ in range(8)] for _ in range(2)]
                    for jg in range(4):
                        hj = hjs[jg % 2]
                        w2_t = w2s[jg % 2]
                        for j8 in range(8):
                            j = 8 * jg + j8
                            w1t = w1p.tile([128, D], F32R, tag="w1")
                            nc.sync.dma_start(
                                w1t[:], w1r[j].rearrange("a b c -> a (b c)"))
                            f1 = psd.tile([128, T], F32, tag="f1", bufs=2,
                                          name="f1")
                            for jj in range(2):
                                sl = slice(512 * jj, 512 * (jj + 1))
                                for c in range(ND):
                                    nc.tensor.matmul(
                                        f1[:, sl],
                                        w1t[:, 128 * c:128 * (c + 1)],
                                        xln2[c][:, sl],
                                        start=(c == 0), stop=(c == ND - 1))
                            nc.vector.tensor_scalar(
                                out=hj[j8][:], in0=f1[:],
                                scalar1=c_b1[:, j:j + 1], scalar2=0.0,
                                op0=ALU.add, op1=ALU.max)
                            nc.sync.dma_start(
                                w2_t[j8][:], w2b[128 * j:128 * (j + 1), :])
                        for m in range(NT):
                            for nb in range(2):
                                sl = slice(512 * nb, 512 * (nb + 1))
                                fb = psd.tile([128, 512], F32, tag="fb",
                                              bufs=4, name="fb")
                                for j8 in range(8):
                                    nc.tensor.matmul(
                                        fb[:],
                                        hj[j8][:, 128 * m:128 * (m + 1)],
                                        w2_t[j8][:, sl],
                                        start=(j8 == 0), stop=(j8 == 7))
                                if jg == 0:
                                    nc.vector.tensor_copy(acc[m][:, sl],
                                                          fb[:])
                                else:
                                    nc.vector.tensor_add(
                                        acc[m][:, sl], fb[:], acc[m][:, sl])
                    for m in range(NT):
                        nc.vector.tensor_add(xp2[m][:], acc[m][:], xp2[m][:])
                        nc.sync.dma_start(
                            out_l[128 * m:128 * (m + 1), :], xp2[m][:])

            if reps > 1:
                with tc.For_i(0, reps, 1):
                    _phases()
            else:
                _phases()
          except _Done:
            pass

    nc.compile()
    return nc


_NC = None


def _get_nc():
    global _NC
    if _NC is None:
        _NC = _build()
    return _NC


def _prep_common(wq, wk, wv, w_proj, b_proj, w1, b1, w2, b2, g1, be1, g2, be2):
    f = np.float32
    wq = np.asarray(wq, f)
    wk = np.asarray(wk, f)
    wv = np.asarray(wv, f)

    def pack_pairs(w):
        w5 = w.reshape(H // 2, 2, ND, 128, E)
        return np.ascontiguousarray(
            w5.transpose(0, 3, 2, 1, 4).reshape(H // 2, 128, ND, 128))

    w1 = np.asarray(w1, f)
    return {
        "wqp": pack_pairs(wq),
        "wkp": pack_pairs(wk),
        "wva": np.ascontiguousarray(wv.transpose(1, 0, 2).reshape(D, D)),
        "wpj": np.ascontiguousarray(np.asarray(w_proj, f)),
        "w1r": np.ascontiguousarray(
            w1.reshape(ND, 128, NJ, 128).transpose(2, 1, 0, 3)),
        "w2b": np.ascontiguousarray(
            np.asarray(w2, f).astype(ml_dtypes.bfloat16)),
        "g1f": np.ascontiguousarray(np.asarray(g1, f).reshape(ND, 128).T),
        "be1f": np.ascontiguousarray(np.asarray(be1, f).reshape(ND, 128).T),
        "g2f": np.ascontiguousarray(np.asarray(g2, f).reshape(ND, 128).T),
        "be2f": np.ascontiguousarray(np.asarray(be2, f).reshape(ND, 128).T),
        "bpf": np.ascontiguousarray(np.asarray(b_proj, f).reshape(ND, 128).T),
        "b1f": np.ascontiguousarray(np.asarray(b1, f).reshape(NJ, 128).T),
        "b2b": np.ascontiguousarray(np.tile(np.asarray(b2, f), (128, 1))),
        "idn": np.eye(128, dtype=f),
        "onz": np.ones((128, 128), f),
        "ond": np.full((128, 1), 1.0 / D, f),
        "mby": np.where(np.arange(128)[None, :] >= np.arange(128)[:, None],
                        1.0, 0.0).astype(ml_dtypes.bfloat16),
    }


def kernel(x, wq, wk, wv, w_proj, b_proj, w1, b1, w2, b2, g1, be1, g2, be2,
           **bench):
    nc = _get_nc()
    common = _prep_common(wq, wk, wv, w_proj, b_proj, w1, b1, w2, b2,
                          g1, be1, g2, be2)
    x = np.asarray(x, np.float32)
    in_maps = [dict(common, x_l=np.ascontiguousarray(x[b]))
               for b in range(NCORE)]
    res = bass_utils.run_bass_kernel_spmd(
        nc, in_maps, core_ids=list(range(NCORE)), **bench)
    out = np.stack([res.results[b]["out_l"] for b in range(NCORE)])
    if bench:
        kernel.last_results = res
    return out


if __name__ == "__main__":
    _build()
    print("built ok")



# revision 11
# speedup vs baseline: 1.1515x; 1.1515x over previous
import sys

sys.path.insert(0, "/opt/trn_rl_repo")

import numpy as np
import ml_dtypes

import concourse.bacc as bacc
import concourse.mybir as mybir
from concourse.tile import TileContext
from concourse import bass_utils

F32 = mybir.dt.float32
F32R = mybir.dt.float32r
BF16 = mybir.dt.bfloat16
AF = mybir.ActivationFunctionType
ALU = mybir.AluOpType

B, T, D = 8, 1024, 1024
H, E = 16, 64
DFF = 4 * D
NCORE = 8
NT = T // 128
ND = D // 128
NJ = DFF // 128
LN_EPS = 1e-5


class _Done(Exception):
    pass


def r32(ap):
    return ap.bitcast(F32R)


def v32(ap):
    return ap.bitcast(F32)


def _build(upto=9, reps=1):
    nc = bacc.Bacc("TRN2", target_bir_lowering=False, debug=False,
                   num_devices=NCORE)

    x_l = nc.dram_tensor("x_l", [T, D], F32R, kind="ExternalInput")
    wqp = nc.dram_tensor("wqp", [H // 2, 128, ND, 128], F32R,
                         kind="ExternalInput")
    wkp = nc.dram_tensor("wkp", [H // 2, 128, ND, 128], F32R,
                         kind="ExternalInput")
    wva = nc.dram_tensor("wva", [D, D], F32R, kind="ExternalInput")
    wpj = nc.dram_tensor("wpj", [D, D], F32R, kind="ExternalInput")
    w1r = nc.dram_tensor("w1r", [NJ, 128, ND, 128], F32R,
                         kind="ExternalInput")
    w2b = nc.dram_tensor("w2b", [DFF, D], BF16, kind="ExternalInput")
    g1f = nc.dram_tensor("g1f", [128, ND], F32, kind="ExternalInput")
    be1f = nc.dram_tensor("be1f", [128, ND], F32, kind="ExternalInput")
    g2f = nc.dram_tensor("g2f", [128, ND], F32, kind="ExternalInput")
    be2f = nc.dram_tensor("be2f", [128, ND], F32, kind="ExternalInput")
    bpf = nc.dram_tensor("bpf", [128, ND], F32, kind="ExternalInput")
    b1f = nc.dram_tensor("b1f", [128, NJ], F32, kind="ExternalInput")
    b2b = nc.dram_tensor("b2b", [128, D], F32, kind="ExternalInput")
    idn = nc.dram_tensor("idn", [128, 128], F32R, kind="ExternalInput")
    onz = nc.dram_tensor("onz", [128, 128], F32R, kind="ExternalInput")
    ond = nc.dram_tensor("ond", [128, 1], F32R, kind="ExternalInput")
    mby = nc.dram_tensor("mby", [128, 128], BF16, kind="ExternalInput")
    out_l = nc.dram_tensor("out_l", [T, D], F32, kind="ExternalOutput")

    def dump_fm(tiles, n):
        for c in range(n):
            nc.sync.dma_start(out_l[128 * c:128 * (c + 1), :],
                              v32(tiles[c][:]))

    with TileContext(nc) as tc:
        with (
            tc.tile_pool(name="fm", bufs=16) as fm,
            tc.tile_pool(name="const", bufs=1) as cp,
        ):
          try:
            def cload(name, dram, shape, dtype=F32):
                t = cp.tile(list(shape), dtype, tag=name, name=name)
                nc.sync.dma_start(t[:], dram[:])
                return t

            c_id = cload("idn", idn, [128, 128], F32R)
            c_idb = cp.tile([128, 128], BF16, tag="idnb", name="idnb")
            nc.vector.tensor_copy(c_idb[:], c_id[:])
            c_on = cload("onz", onz, [128, 128], F32R)
            c_od = cload("ond", ond, [128, 1], F32R)
            c_g1 = cload("g1f", g1f, [128, ND])
            c_be1 = cload("be1f", be1f, [128, ND])
            c_g2 = cload("g2f", g2f, [128, ND])
            c_be2 = cload("be2f", be2f, [128, ND])
            c_bp = cload("bpf", bpf, [128, ND])
            c_b1 = cload("b1f", b1f, [128, NJ])
            c_b2 = cload("b2b", b2b, [128, D])
            c_mb = cload("mby", mby, [128, 128], BF16)
            c_eps = cp.tile([128, 1], F32, tag="eps", name="eps")
            nc.gpsimd.memset(c_eps[:], LN_EPS)

            def _phases():
                xf = [fm.tile([128, T], F32R, tag="fm", name="xf")
                      for _ in range(ND)]
                qT, kT, v_sb = [], [], []
                with (
                    tc.tile_pool(name="xin", bufs=2) as xp,
                    tc.tile_pool(name="scr", bufs=2) as scrp,
                    tc.tile_pool(name="st1", bufs=4) as st1,
                    tc.tile_pool(name="ps_a", bufs=1, space="PSUM") as psa,
                ):
                    for m in range(NT):
                        xm = xp.tile([128, D], F32R, tag="xin")
                        nc.sync.dma_start(xm[:], x_l[128 * m:128 * (m + 1), :])
                        scr = scrp.tile([128, D], BF16, tag="scr",
                                        name="scr")
                        st = st1.tile([128, 4], F32, tag="st", name="st")
                        nc.scalar.activation(scr[:], xm[:], AF.Square,
                                             accum_out=st[:, 1:2])
                        nc.scalar.activation(scr[:], xm[:], AF.Identity,
                                             accum_out=st[:, 0:1])
                        nc.vector.tensor_scalar_mul(st[:, 0:1], st[:, 0:1],
                                                    1.0 / D)
                        nc.vector.tensor_scalar_mul(st[:, 1:2], st[:, 1:2],
                                                    1.0 / D)
                        nc.vector.tensor_mul(st[:, 2:3], st[:, 0:1],
                                             st[:, 0:1])
                        nc.vector.tensor_sub(st[:, 2:3], st[:, 1:2],
                                             st[:, 2:3])
                        nc.scalar.activation(st[:, 2:3], st[:, 2:3], AF.Sqrt,
                                             bias=c_eps[:])
                        nc.vector.reciprocal(st[:, 3:4], st[:, 2:3])
                        nc.vector.tensor_scalar(
                            out=xm[:], in0=xm[:], scalar1=st[:, 0:1],
                            scalar2=st[:, 3:4], op0=ALU.subtract, op1=ALU.mult)
                        for c in range(ND):
                            pt = psa.tile([128, 128], F32R, tag="tr",
                                          bufs=3, name="pt")
                            nc.tensor.transpose(
                                pt[:], xm[:, 128 * c:128 * (c + 1)], c_id[:])
                            nc.vector.tensor_scalar(
                                out=xf[c][:, 128 * m:128 * (m + 1)],
                                in0=pt[:], scalar1=c_g1[:, c:c + 1],
                                scalar2=c_be1[:, c:c + 1],
                                op0=ALU.mult, op1=ALU.add)
                if upto == 1:
                    dump_fm(xf, ND)
                    raise _Done()
                xln1 = xf

                at_t = None
                with (
                    tc.tile_pool(name="qk", bufs=8) as qkp,
                    tc.tile_pool(name="vsb", bufs=8) as vp,
                    tc.tile_pool(name="atm", bufs=8) as atp,
                    tc.tile_pool(name="se", bufs=18) as sep,
                    tc.tile_pool(name="rc", bufs=8) as rcp,
                    tc.tile_pool(name="wv", bufs=8) as wvpool,
                    tc.tile_pool(name="wq", bufs=2) as wqpool,
                    tc.tile_pool(name="wk", bufs=2) as wkpool,
                    tc.tile_pool(name="ps_b", bufs=1, space="PSUM") as psb,
                ):
                    at_t = [atp.tile([128, D], BF16, tag="atm", name="at_t")
                            for _ in range(NT)]

                    def load_wv_half(nb):
                        ws = []
                        for c in range(ND):
                            w = wvpool.tile([128, 512], F32R, tag="wv",
                                            name="wv_t")
                            nc.sync.dma_start(
                                w[:], wva[128 * c:128 * (c + 1),
                                          512 * nb:512 * (nb + 1)])
                            ws.append(w)
                        return ws

                    def emit_qk_pair(p):
                        wq_t = wqpool.tile([128, D], F32R, tag="wq")
                        nc.sync.dma_start(
                            wq_t[:], wqp[p].rearrange("a b c -> a (b c)"))
                        wk_t = wkpool.tile([128, D], F32R, tag="wk")
                        nc.sync.dma_start(
                            wk_t[:], wkp[p].rearrange("a b c -> a (b c)"))
                        q_t = qkp.tile([128, T], BF16, tag="qk")
                        k_t = qkp.tile([128, T], BF16, tag="qk")
                        for dst, w_t in ((q_t, wq_t), (k_t, wk_t)):
                            for jj in range(2):
                                sl = slice(512 * jj, 512 * (jj + 1))
                                ps = psb.tile([128, 512], F32, tag="qkp",
                                              bufs=2, name="psqk")
                                for c in range(ND):
                                    nc.tensor.matmul(
                                        ps[:],
                                        w_t[:, 128 * c:128 * (c + 1)],
                                        xln1[c][:, sl],
                                        start=(c == 0),
                                        stop=(c == ND - 1))
                                nc.vector.tensor_copy(dst[:, sl], ps[:])
                        qT.append(q_t)
                        kT.append(k_t)

                    def emit_vproj_half(i, nb, wv_t):
                        if nb == 0:
                            vt = vp.tile([128, H * 65], BF16, tag="v",
                                         name="vt")
                            v_sb.append(vt)
                        vt = v_sb[i]
                        v3 = vt.rearrange("p (h e) -> p h e", e=65)
                        if nb == 0:
                            nc.gpsimd.memset(v3[:, :, 64:65], 1.0)
                        ps = psb.tile([128, 512], F32, tag="av",
                                      bufs=2, name="psv")
                        for c in range(ND):
                            nc.tensor.matmul(
                                ps[:],
                                xln1[c][:, 128 * i:128 * (i + 1)],
                                wv_t[c][:],
                                start=(c == 0), stop=(c == ND - 1))
                        nc.vector.tensor_copy(
                            v3[:, 8 * nb:8 * (nb + 1), 0:64],
                            ps[:, 0:512].rearrange("p (h e) -> p h e", e=64))

                    def emit_scores_blk(h, i, se):
                        p, q = h // 2, h % 2
                        qsl = slice(64 * q, 64 * (q + 1))
                        ib = slice(128 * i, 128 * (i + 1))
                        lo = 128 * i
                        sc = psb.tile([128, T], F32, tag="sc", bufs=2,
                                      name="sc")
                        if i < 4:
                            nc.tensor.matmul(
                                sc[:, lo:512], kT[p][qsl, ib],
                                qT[p][qsl, lo:512], start=True, stop=True)
                        nc.tensor.matmul(
                            sc[:, 512:], kT[p][qsl, ib],
                            qT[p][qsl, 512:], start=True, stop=True)
                        st = sep.tile([128, T], BF16, tag="se", name="se")
                        nc.scalar.activation(st[:, lo:], sc[:, lo:],
                                             AF.Exp, scale=0.125)
                        nc.vector.tensor_mul(st[:, ib], st[:, ib], c_mb[:])
                        se.append(st)

                    def emit_av_blk(h, m, se):
                        av = psb.tile([128, 65], F32, tag="av", bufs=2,
                                      name="av")
                        for i in range(m + 1):
                            nc.tensor.matmul(
                                av[:],
                                se[i][:, 128 * m:128 * (m + 1)],
                                v_sb[i].rearrange(
                                    "p (h e) -> p h e", e=65)[:, h, :],
                                start=(i == 0), stop=(i == m))
                        rcol = rcp.tile([128, 1], F32, tag="rc", name="rc")
                        nc.vector.reciprocal_approx_fast(
                            rcol[:], av[:, 64:65])
                        nc.vector.tensor_scalar_mul(
                            at_t[m][:, 64 * h:64 * (h + 1)],
                            av[:, 0:64], rcol[:])

                    se_prev = None
                    for h in range(H):
                        p, q = h // 2, h % 2
                        if q == 0:
                            emit_qk_pair(p)
                        if h == 1:
                            wvh = load_wv_half(0)
                            for i in range(NT):
                                emit_vproj_half(i, 0, wvh)
                        if h == 3:
                            wvh = load_wv_half(1)
                            for i in range(NT):
                                emit_vproj_half(i, 1, wvh)
                        se_cur = []
                        for k in range(NT):
                            emit_scores_blk(h, k, se_cur)
                            if se_prev is not None:
                                emit_av_blk(h - 1, k, se_prev)
                        se_prev = se_cur
                    for k in range(NT):
                        emit_av_blk(H - 1, k, se_prev)

                    if upto == 2:
                        for c in range(2):
                            nc.sync.dma_start(
                                out_l[128 * c:128 * (c + 1), :],
                                v32(qT[c][:]))
                            nc.sync.dma_start(
                                out_l[128 * (c + 2):128 * (c + 3), :],
                                v32(kT[c][:]))
                        raise _Done()

                    aoT = [fm.tile([128, T], F32R, tag="fm", name="aoT")
                           for _ in range(ND)]
                    for m in range(NT):
                        for c in range(ND):
                            pt = psb.tile([128, 128], BF16, tag="qkp",
                                          bufs=2, name="pt2")
                            nc.tensor.transpose(
                                pt[:], at_t[m][:, 128 * c:128 * (c + 1)],
                                c_idb[:])
                            nc.vector.tensor_copy(
                                aoT[c][:, 128 * m:128 * (m + 1)], pt[:])
                if upto == 3:
                    dump_fm(aoT, ND)
                    raise _Done()

                with (
                    tc.tile_pool(name="wpj", bufs=8) as wpjp,
                    tc.tile_pool(name="sq2", bufs=3) as sq2,
                    tc.tile_pool(name="rw2", bufs=1) as rw2,
                    tc.tile_pool(name="ps_c", bufs=1, space="PSUM") as psc,
                ):
                    wp_t = []
                    for c in range(ND):
                        w = wpjp.tile([128, D], F32R, tag="wpj")
                        nc.sync.dma_start(
                            w[:], wpj[128 * c:128 * (c + 1), :])
                        wp_t.append(w)
                    mu_r = rw2.tile([1, T], F32, tag="mu_r")
                    ms_r = rw2.tile([1, T], F32, tag="ms_r")
                    sd_r = rw2.tile([1, T], F32, tag="sd_r")
                    rc_r = rw2.tile([1, T], F32, tag="rc_r")
                    mu_b = rw2.tile([128, T], F32R, tag="mu_b")
                    r_b = rw2.tile([128, T], F32R, tag="r_b")
                    xp2 = [fm.tile([128, D], F32, tag="fm", name="xp2")
                           for _ in range(NT)]
                    for jj in range(2):
                        sl = slice(512 * jj, 512 * (jj + 1))
                        for co in range(ND):
                            ps = psc.tile([128, 512], F32, tag="pj",
                                          bufs=2, name="pspj")
                            for c in range(ND):
                                nc.tensor.matmul(
                                    ps[:],
                                    wp_t[c][:, 128 * co:128 * (co + 1)],
                                    aoT[c][:, sl],
                                    start=(c == 0), stop=(c == ND - 1))
                            nc.vector.scalar_tensor_tensor(
                                out=xln1[co][:, sl], in0=ps[:],
                                scalar=c_bp[:, co:co + 1],
                                in1=xln1[co][:, sl],
                                op0=ALU.add, op1=ALU.add)
                        sq = []
                        for c in range(ND):
                            s = sq2.tile([128, 512], F32R, tag="sq",
                                         name="sq")
                            nc.vector.tensor_mul(s[:], xln1[c][:, sl],
                                                 xln1[c][:, sl])
                            sq.append(s)
                        st_x = psc.tile([1, 512], F32, tag="st2", bufs=2,
                                        name="st_x")
                        st_q = psc.tile([1, 512], F32, tag="st2", bufs=2,
                                        name="st_q")
                        for c in range(ND):
                            nc.tensor.matmul(
                                st_x[:], c_od[:], xln1[c][:, sl],
                                start=(c == 0), stop=(c == ND - 1))
                        for c in range(ND):
                            nc.tensor.matmul(
                                st_q[:], c_od[:], sq[c][:],
                                start=(c == 0), stop=(c == ND - 1))
                        nc.vector.tensor_copy(mu_r[:, sl], st_x[:])
                        nc.vector.tensor_copy(ms_r[:, sl], st_q[:])
                        nc.vector.tensor_mul(sd_r[:, sl], mu_r[:, sl],
                                             mu_r[:, sl])
                        nc.vector.tensor_sub(sd_r[:, sl], ms_r[:, sl],
                                             sd_r[:, sl])
                        nc.scalar.activation(sd_r[:, sl], sd_r[:, sl],
                                             AF.Sqrt, bias=c_eps[0:1, :])
                        nc.vector.reciprocal_approx_fast(rc_r[:, sl],
                                                         sd_r[:, sl])
                        pm = psc.tile([128, 512], F32, tag="pj", bufs=2,
                                      name="pm")
                        nc.tensor.matmul(pm[:], v32(c_on[0:1, :]),
                                         mu_r[:, sl])
                        nc.vector.tensor_copy(mu_b[:, sl], pm[:])
                        pr = psc.tile([128, 512], F32, tag="pj", bufs=2,
                                      name="pr")
                        nc.tensor.matmul(pr[:], v32(c_on[0:1, :]),
                                         rc_r[:, sl])
                        nc.vector.tensor_copy(r_b[:, sl], pr[:])
                        for c in range(ND):
                            nc.vector.tensor_sub(xln1[c][:, sl],
                                                 xln1[c][:, sl],
                                                 mu_b[:, sl])
                            nc.vector.tensor_mul(xln1[c][:, sl],
                                                 xln1[c][:, sl],
                                                 r_b[:, sl])
                            nc.vector.tensor_scalar(
                                out=xln1[c][:, sl], in0=xln1[c][:, sl],
                                scalar1=c_g2[:, c:c + 1],
                                scalar2=c_be2[:, c:c + 1],
                                op0=ALU.mult, op1=ALU.add)
                        for m in range(4 * jj, 4 * (jj + 1)):
                            for c in range(ND):
                                cs = slice(128 * c, 128 * (c + 1))
                                pt = psc.tile([128, 128], F32R, tag="tr3",
                                              bufs=2, name="pt3")
                                nc.tensor.transpose(
                                    pt[:],
                                    xln1[c][:, 128 * m:128 * (m + 1)],
                                    c_id[:])
                                nc.vector.tensor_add(
                                    xp2[m][:, cs], pt[:], c_b2[:, cs])
                    xln2 = xln1
                    if upto == 5:
                        dump_fm(xln2, ND)
                        raise _Done()

                with (
                    tc.tile_pool(name="acc", bufs=8) as accp,
                    tc.tile_pool(name="hj", bufs=16) as hjp,
                    tc.tile_pool(name="w1t", bufs=3) as w1p,
                    tc.tile_pool(name="w2t", bufs=16) as w2p,
                    tc.tile_pool(name="ps_d", bufs=1, space="PSUM") as psd,
                ):
                    acc = [accp.tile([128, D], F32, tag="acc", name="acc")
                           for _ in range(NT)]
                    hjs = [[hjp.tile([128, T], BF16, tag="hj", name="hj")
                            for _ in range(8)] for _ in range(2)]
                    w2s = [[w2p.tile([128, D], BF16, tag="w2", name="w2t")
                            for _ in range(8)] for _ in range(2)]
                    for jg in range(4):
                        hj = hjs[jg % 2]
                        w2_t = w2s[jg % 2]
                        for j8 in range(8):
                            j = 8 * jg + j8
                            w1t = w1p.tile([128, D], F32R, tag="w1")
                            nc.sync.dma_start(
                                w1t[:], w1r[j].rearrange("a b c -> a (b c)"))
                            f1 = psd.tile([128, T], F32, tag="f1", bufs=2,
                                          name="f1")
                            for jj in range(2):
                                sl = slice(512 * jj, 512 * (jj + 1))
                                for c in range(ND):
                                    nc.tensor.matmul(
                                        f1[:, sl],
                                        w1t[:, 128 * c:128 * (c + 1)],
                                        xln2[c][:, sl],
                                        start=(c == 0), stop=(c == ND - 1))
                            nc.vector.tensor_scalar(
                                out=hj[j8][:], in0=f1[:],
                                scalar1=c_b1[:, j:j + 1], scalar2=0.0,
                                op0=ALU.add, op1=ALU.max)
                            nc.sync.dma_start(
                                w2_t[j8][:], w2b[128 * j:128 * (j + 1), :])
                        for m in range(NT):
                            for nb in range(2):
                                sl = slice(512 * nb, 512 * (nb + 1))
                                fb = psd.tile([128, 512], F32, tag="fb",
                                              bufs=4, name="fb")
                                for j8 in range(8):
                                    nc.tensor.matmul(
                                        fb[:],
                                        hj[j8][:, 128 * m:128 * (m + 1)],
                                        w2_t[j8][:, sl],
                                        start=(j8 == 0), stop=(j8 == 7))
                                if jg == 0:
                                    nc.vector.tensor_copy(acc[m][:, sl],
                                                          fb[:])
                                else:
                                    nc.vector.tensor_add(
                                        acc[m][:, sl], fb[:], acc[m][:, sl])
                    for m in range(NT):
                        nc.vector.tensor_add(xp2[m][:], acc[m][:], xp2[m][:])
                        nc.sync.dma_start(
                            out_l[128 * m:128 * (m + 1), :], xp2[m][:])

            if reps > 1:
                with tc.For_i(0, reps, 1):
                    _phases()
            else:
                _phases()
          except _Done:
            pass

    nc.compile()
    return nc


_NC = None


def _get_nc():
    global _NC
    if _NC is None:
        _NC = _build()
    return _NC


def _prep_common(wq, wk, wv, w_proj, b_proj, w1, b1, w2, b2, g1, be1, g2, be2):
    f = np.float32
    wq = np.asarray(wq, f)
    wk = np.asarray(wk, f)
    wv = np.asarray(wv, f)

    def pack_pairs(w):
        w5 = w.reshape(H // 2, 2, ND, 128, E)
        return np.ascontiguousarray(
            w5.transpose(0, 3, 2, 1, 4).reshape(H // 2, 128, ND, 128))

    w1 = np.asarray(w1, f)
    return {
        "wqp": pack_pairs(wq),
        "wkp": pack_pairs(wk),
        "wva": np.ascontiguousarray(wv.transpose(1, 0, 2).reshape(D, D)),
        "wpj": np.ascontiguousarray(np.asarray(w_proj, f)),
        "w1r": np.ascontiguousarray(
            w1.reshape(ND, 128, NJ, 128).transpose(2, 1, 0, 3)),
        "w2b": np.ascontiguousarray(
            np.asarray(w2, f).astype(ml_dtypes.bfloat16)),
        "g1f": np.ascontiguousarray(np.asarray(g1, f).reshape(ND, 128).T),
        "be1f": np.ascontiguousarray(np.asarray(be1, f).reshape(ND, 128).T),
        "g2f": np.ascontiguousarray(np.asarray(g2, f).reshape(ND, 128).T),
        "be2f": np.ascontiguousarray(np.asarray(be2, f).reshape(ND, 128).T),
        "bpf": np.ascontiguousarray(np.asarray(b_proj, f).reshape(ND, 128).T),
        "b1f": np.ascontiguousarray(np.asarray(b1, f).reshape(NJ, 128).T),
        "b2b": np.ascontiguousarray(np.tile(np.asarray(b2, f), (128, 1))),
        "idn": np.eye(128, dtype=f),
        "onz": np.ones((128, 128), f),
        "ond": np.full((128, 1), 1.0 / D, f),
        "mby": np.where(np.arange(128)[None, :] >= np.arange(128)[:, None],
                        1.0, 0.0).astype(ml_dtypes.bfloat16),
    }


def kernel(x, wq, wk, wv, w_proj, b_proj, w1, b1, w2, b2, g1, be1, g2, be2,
           **bench):
    nc = _get_nc()
    common = _prep_common(wq, wk, wv, w_proj, b_proj, w1, b1, w2, b2,
                          g1, be1, g2, be2)
    x = np.asarray(x, np.float32)
    in_maps = [dict(common, x_l=np.ascontiguousarray(x[b]))
               for b in range(NCORE)]
    res = bass_utils.run_bass_kernel_spmd(
        nc, in_maps, core_ids=list(range(NCORE)), **bench)
    out = np.stack([res.results[b]["out_l"] for b in range(NCORE)])
    if bench:
        kernel.last_results = res
    return out


if __name__ == "__main__":
    _build()
    print("built ok")


# revision 15
# speedup vs baseline: 1.2206x; 1.0600x over previous
import sys

sys.path.insert(0, "/opt/trn_rl_repo")

import numpy as np
import ml_dtypes

import concourse.bacc as bacc
import concourse.mybir as mybir
from concourse.tile import TileContext
from concourse import bass_utils

F32 = mybir.dt.float32
F32R = mybir.dt.float32r
BF16 = mybir.dt.bfloat16
AF = mybir.ActivationFunctionType
ALU = mybir.AluOpType

B, T, D = 8, 1024, 1024
H, E = 16, 64
DFF = 4 * D
NCORE = 8
NT = T // 128
ND = D // 128
NJ = DFF // 128
LN_EPS = 1e-5


class _Done(Exception):
    pass


def r32(ap):
    return ap.bitcast(F32R)


def v32(ap):
    return ap.bitcast(F32)


def _build(upto=9, reps=1):
    nc = bacc.Bacc("TRN2", target_bir_lowering=False, debug=False,
                   num_devices=NCORE)

    x_l = nc.dram_tensor("x_l", [T, D], F32R, kind="ExternalInput")
    wqp = nc.dram_tensor("wqp", [H // 2, 128, ND, 128], BF16,
                         kind="ExternalInput")
    wkp = nc.dram_tensor("wkp", [H // 2, 128, ND, 128], BF16,
                         kind="ExternalInput")
    wva = nc.dram_tensor("wva", [D, D], BF16, kind="ExternalInput")
    wpj = nc.dram_tensor("wpj", [D, D], BF16, kind="ExternalInput")
    w1r = nc.dram_tensor("w1r", [NJ, 128, ND, 128], BF16,
                         kind="ExternalInput")
    w2b = nc.dram_tensor("w2b", [DFF, D], BF16, kind="ExternalInput")
    g1f = nc.dram_tensor("g1f", [128, ND], F32, kind="ExternalInput")
    be1f = nc.dram_tensor("be1f", [128, ND], F32, kind="ExternalInput")
    g2f = nc.dram_tensor("g2f", [128, ND], F32, kind="ExternalInput")
    be2f = nc.dram_tensor("be2f", [128, ND], F32, kind="ExternalInput")
    bpf = nc.dram_tensor("bpf", [128, ND], F32, kind="ExternalInput")
    b1f = nc.dram_tensor("b1f", [128, NJ], F32, kind="ExternalInput")
    b2b = nc.dram_tensor("b2b", [128, D], F32, kind="ExternalInput")
    idn = nc.dram_tensor("idn", [128, 128], F32R, kind="ExternalInput")
    onz = nc.dram_tensor("onz", [128, 128], F32R, kind="ExternalInput")
    ond = nc.dram_tensor("ond", [128, 1], F32R, kind="ExternalInput")
    mby = nc.dram_tensor("mby", [128, 128], BF16, kind="ExternalInput")
    out_l = nc.dram_tensor("out_l", [T, D], F32, kind="ExternalOutput")

    def dump_fm(tiles, n):
        for c in range(n):
            nc.sync.dma_start(out_l[128 * c:128 * (c + 1), :],
                              v32(tiles[c][:]))

    with TileContext(nc) as tc:
        with (
            tc.tile_pool(name="fm", bufs=16) as fm,
            tc.tile_pool(name="const", bufs=1) as cp,
        ):
          try:
            def cload(name, dram, shape, dtype=F32):
                t = cp.tile(list(shape), dtype, tag=name, name=name)
                nc.sync.dma_start(t[:], dram[:])
                return t

            c_id = cload("idn", idn, [128, 128], F32R)
            c_idb = cp.tile([128, 128], BF16, tag="idnb", name="idnb")
            nc.vector.tensor_copy(c_idb[:], c_id[:])
            c_on = cload("onz", onz, [128, 128], F32R)
            c_od = cload("ond", ond, [128, 1], F32R)
            c_g1 = cload("g1f", g1f, [128, ND])
            c_be1 = cload("be1f", be1f, [128, ND])
            c_g2 = cload("g2f", g2f, [128, ND])
            c_be2 = cload("be2f", be2f, [128, ND])
            c_bp = cload("bpf", bpf, [128, ND])
            c_b1 = cload("b1f", b1f, [128, NJ])
            c_b2 = cload("b2b", b2b, [128, D])
            c_mb = cload("mby", mby, [128, 128], BF16)
            c_eps = cp.tile([128, 1], F32, tag="eps", name="eps")
            nc.gpsimd.memset(c_eps[:], LN_EPS)

            def _phases():
                xf = [fm.tile([128, T], F32R, tag="fm", name="xf")
                      for _ in range(ND)]
                qT, kT, v_sb = [], [], []
                with tc.tile_pool(name="xb", bufs=8) as xbp:
                  xb = [xbp.tile([128, T], BF16, tag="xb", name="xb")
                        for _ in range(ND)]
                  with tc.tile_pool(name="aoTp", bufs=8) as aop:
                    aoT = [aop.tile([128, T], BF16, tag="aoT", name="aoT")
                           for _ in range(ND)]
                    with (
                        tc.tile_pool(name="atm", bufs=8) as atp,
                        tc.tile_pool(name="xin", bufs=2) as xp,
                        tc.tile_pool(name="scr", bufs=1) as scrp,
                        tc.tile_pool(name="st1", bufs=4) as st1,
                        tc.tile_pool(name="qk", bufs=5) as qkp,
                        tc.tile_pool(name="vsb", bufs=8) as vp,
                        tc.tile_pool(name="se", bufs=16) as sep,
                        tc.tile_pool(name="rc", bufs=8) as rcp,
                        tc.tile_pool(name="wv", bufs=8) as wvpool,
                        tc.tile_pool(name="wq", bufs=2) as wqpool,
                        tc.tile_pool(name="wk", bufs=2) as wkpool,
                        tc.tile_pool(name="ps_b", bufs=1, space="PSUM")
                        as psb,
                    ):
                        at_t = [atp.tile([128, D], BF16, tag="atm",
                                         name="at_t") for _ in range(NT)]
                        qk_w = {}

                        def emit_qk_half(p, jj):
                            if jj == 0:
                                wq_t = wqpool.tile([128, D], BF16, tag="wq")
                                nc.sync.dma_start(
                                    wq_t[:],
                                    wqp[p].rearrange("a b c -> a (b c)"))
                                wk_t = wkpool.tile([128, D], BF16, tag="wk")
                                nc.sync.dma_start(
                                    wk_t[:],
                                    wkp[p].rearrange("a b c -> a (b c)"))
                                qk_w[p] = (wq_t, wk_t)
                                qT.append(qkp.tile([128, T], BF16, tag="qk", name="q_t"))
                                kT.append(qkp.tile([128, T], BF16, tag="qk", name="k_t"))
                            wq_t, wk_t = qk_w[p]
                            sl = slice(512 * jj, 512 * (jj + 1))
                            for dst, w_t in ((qT[p], wq_t), (kT[p], wk_t)):
                                ps = psb.tile([128, 512], F32, tag="qkp",
                                              bufs=2, name="psqk")
                                for c in range(ND):
                                    nc.tensor.matmul(
                                        ps[:],
                                        w_t[:, 128 * c:128 * (c + 1)],
                                        xb[c][:, sl],
                                        start=(c == 0),
                                        stop=(c == ND - 1))
                                nc.vector.tensor_copy(dst[:, sl], ps[:])

                        for m in range(NT):
                            xm = xp.tile([128, D], F32R, tag="xin")
                            nc.sync.dma_start(
                                xm[:], x_l[128 * m:128 * (m + 1), :])
                            scr = scrp.tile([128, D], BF16, tag="scr",
                                            name="scr")
                            st = st1.tile([128, 4], F32, tag="st", name="st")
                            nc.scalar.activation(scr[:], xm[:], AF.Square,
                                                 accum_out=st[:, 1:2])
                            nc.scalar.activation(scr[:], xm[:], AF.Identity,
                                                 accum_out=st[:, 0:1])
                            nc.vector.tensor_scalar_mul(st[:, 0:1],
                                                        st[:, 0:1], 1.0 / D)
                            nc.vector.tensor_scalar_mul(st[:, 1:2],
                                                        st[:, 1:2], 1.0 / D)
                            nc.vector.tensor_mul(st[:, 2:3], st[:, 0:1],
                                                 st[:, 0:1])
                            nc.vector.tensor_sub(st[:, 2:3], st[:, 1:2],
                                                 st[:, 2:3])
                            nc.scalar.activation(st[:, 2:3], st[:, 2:3],
                                                 AF.Sqrt, bias=c_eps[:])
                            nc.vector.reciprocal(st[:, 3:4], st[:, 2:3])
                            nc.vector.tensor_scalar(
                                out=xm[:], in0=xm[:], scalar1=st[:, 0:1],
                                scalar2=st[:, 3:4], op0=ALU.subtract,
                                op1=ALU.mult)
                            for c in range(ND):
                                pt = psb.tile([128, 128], F32R, tag="sc",
                                              bufs=2, name="pt")
                                nc.tensor.transpose(
                                    pt[:], xm[:, 128 * c:128 * (c + 1)],
                                    c_id[:])
                                nc.vector.tensor_scalar(
                                    out=xf[c][:, 128 * m:128 * (m + 1)],
                                    in0=pt[:], scalar1=c_g1[:, c:c + 1],
                                    scalar2=c_be1[:, c:c + 1],
                                    op0=ALU.mult, op1=ALU.add)
                            if m == 3 or m == 7:
                                jj = m // 4
                                sl = slice(512 * jj, 512 * (jj + 1))
                                for c in range(ND):
                                    nc.vector.tensor_copy(xb[c][:, sl],
                                                          xf[c][:, sl])
                                if m == 3:
                                    emit_qk_half(0, 0)
                                    emit_qk_half(1, 0)
                        if upto == 1:
                            dump_fm(xf, ND)
                            raise _Done()
                        xln1 = xf

                        def load_wv_half(nb):
                            ws = []
                            for c in range(ND):
                                w = wvpool.tile([128, 512], BF16, tag="wv",
                                                name="wv_t")
                                nc.sync.dma_start(
                                    w[:], wva[128 * c:128 * (c + 1),
                                              512 * nb:512 * (nb + 1)])
                                ws.append(w)
                            return ws

                        def emit_vproj_half(i, nb, wv_t):
                            if nb == 0:
                                vt = vp.tile([128, H * 65], BF16, tag="v",
                                             name="vt")
                                v_sb.append(vt)
                            vt = v_sb[i]
                            v3 = vt.rearrange("p (h e) -> p h e", e=65)
                            if nb == 0:
                                nc.gpsimd.memset(v3[:, :, 64:65], 1.0)
                            ps = psb.tile([128, 512], F32, tag="av",
                                          bufs=2, name="psv")
                            for c in range(ND):
                                nc.tensor.matmul(
                                    ps[:],
                                    xb[c][:, 128 * i:128 * (i + 1)],
                                    wv_t[c][:],
                                    start=(c == 0), stop=(c == ND - 1))
                            nc.vector.tensor_copy(
                                v3[:, 8 * nb:8 * (nb + 1), 0:64],
                                ps[:, 0:512].rearrange(
                                    "p (h e) -> p h e", e=64))

                        def emit_scores_blk(h, i, se):
                            p, q = h // 2, h % 2
                            qsl = slice(64 * q, 64 * (q + 1))
                            ib = slice(128 * i, 128 * (i + 1))
                            lo = 128 * i
                            sc = psb.tile([128, T], F32, tag="sc", bufs=2,
                                          name="sc")
                            if i < 4:
                                nc.tensor.matmul(
                                    sc[:, lo:512], kT[p][qsl, ib],
                                    qT[p][qsl, lo:512], start=True,
                                    stop=True)
                            nc.tensor.matmul(
                                sc[:, 512:], kT[p][qsl, ib],
                                qT[p][qsl, 512:], start=True, stop=True)
                            st = sep.tile([128, T], BF16, tag="se",
                                          name="se")
                            nc.scalar.activation(st[:, lo:], sc[:, lo:],
                                                 AF.Exp, scale=0.125)
                            nc.vector.tensor_mul(st[:, ib], st[:, ib],
                                                 c_mb[:])
                            se.append(st)

                        def emit_av_blk(h, m, se):
                            av = psb.tile([128, 65], F32, tag="av", bufs=2,
                                          name="av")
                            for i in range(m + 1):
                                nc.tensor.matmul(
                                    av[:],
                                    se[i][:, 128 * m:128 * (m + 1)],
                                    v_sb[i].rearrange(
                                        "p (h e) -> p h e", e=65)[:, h, :],
                                    start=(i == 0), stop=(i == m))
                            rcol = rcp.tile([128, 1], F32, tag="rc",
                                            name="rc")
                            nc.vector.reciprocal_approx_fast(
                                rcol[:], av[:, 64:65])
                            nc.vector.tensor_scalar_mul(
                                at_t[m][:, 64 * h:64 * (h + 1)],
                                av[:, 0:64], rcol[:])

                        se_prev = None
                        for h in range(H):
                            p, q = h // 2, h % 2
                            if q == 0:
                                if p < 2:
                                    emit_qk_half(p, 1)
                                else:
                                    emit_qk_half(p, 0)
                                    emit_qk_half(p, 1)
                            if h == 1:
                                wvh = load_wv_half(0)
                                for i in range(NT):
                                    emit_vproj_half(i, 0, wvh)
                            if h == 3:
                                wvh = load_wv_half(1)
                                for i in range(NT):
                                    emit_vproj_half(i, 1, wvh)
                            se_cur = []
                            for k in range(NT):
                                emit_scores_blk(h, k, se_cur)
                                if se_prev is not None:
                                    emit_av_blk(h - 1, k, se_prev)
                            se_prev = se_cur
                        for k in range(NT):
                            emit_av_blk(H - 1, k, se_prev)

                        if upto == 2:
                            for c in range(2):
                                nc.sync.dma_start(
                                    out_l[128 * c:128 * (c + 1), :],
                                    v32(qT[c][:]))
                                nc.sync.dma_start(
                                    out_l[128 * (c + 2):128 * (c + 3), :],
                                    v32(kT[c][:]))
                            raise _Done()

                        for m in range(NT):
                            for c in range(ND):
                                pt = psb.tile([128, 128], BF16, tag="qkp",
                                              bufs=2, name="pt2")
                                nc.tensor.transpose(
                                    pt[:],
                                    at_t[m][:, 128 * c:128 * (c + 1)],
                                    c_idb[:])
                                nc.vector.tensor_copy(
                                    aoT[c][:, 128 * m:128 * (m + 1)], pt[:])
                    if upto == 3:
                        dump_fm(aoT, ND)
                        raise _Done()

                    with (
                        tc.tile_pool(name="wpj", bufs=8) as wpjp,
                        tc.tile_pool(name="sq2", bufs=3) as sq2,
                        tc.tile_pool(name="rw2", bufs=1) as rw2,
                        tc.tile_pool(name="ps_c", bufs=1, space="PSUM")
                        as psc,
                    ):
                        wp_t = []
                        for c in range(ND):
                            w = wpjp.tile([128, D], BF16, tag="wpj")
                            nc.sync.dma_start(
                                w[:], wpj[128 * c:128 * (c + 1), :])
                            wp_t.append(w)
                        mu_r = rw2.tile([1, T], F32, tag="mu_r")
                        ms_r = rw2.tile([1, T], F32, tag="ms_r")
                        sd_r = rw2.tile([1, T], F32, tag="sd_r")
                        rc_r = rw2.tile([1, T], F32, tag="rc_r")
                        mu_b = rw2.tile([128, T], F32R, tag="mu_b")
                        r_b = rw2.tile([128, T], F32R, tag="r_b")
                        xp2 = [fm.tile([128, D], F32, tag="fm", name="xp2")
                               for _ in range(NT)]
                        for jj in range(2):
                            sl = slice(512 * jj, 512 * (jj + 1))
                            for co in range(ND):
                                ps = psc.tile([128, 512], F32, tag="pj",
                                              bufs=2, name="pspj")
                                for c in range(ND):
                                    nc.tensor.matmul(
                                        ps[:],
                                        wp_t[c][:, 128 * co:128 * (co + 1)],
                                        aoT[c][:, sl],
                                        start=(c == 0), stop=(c == ND - 1))
                                nc.vector.scalar_tensor_tensor(
                                    out=xln1[co][:, sl], in0=ps[:],
                                    scalar=c_bp[:, co:co + 1],
                                    in1=xln1[co][:, sl],
                                    op0=ALU.add, op1=ALU.add)
                            sq = []
                            for c in range(ND):
                                s = sq2.tile([128, 512], F32R, tag="sq",
                                             name="sq")
                                nc.scalar.activation(s[:],
                                                     v32(xln1[c][:, sl]),
                                                     AF.Square)
                                sq.append(s)
                            st_x = psc.tile([1, 512], F32, tag="st2",
                                            bufs=2, name="st_x")
                            st_q = psc.tile([1, 512], F32, tag="st2",
                                            bufs=2, name="st_q")
                            for c in range(ND):
                                nc.tensor.matmul(
                                    st_x[:], c_od[:], xln1[c][:, sl],
                                    start=(c == 0), stop=(c == ND - 1))
                            for c in range(ND):
                                nc.tensor.matmul(
                                    st_q[:], c_od[:], sq[c][:],
                                    start=(c == 0), stop=(c == ND - 1))
                            nc.scalar.copy(mu_r[:, sl], st_x[:])
                            nc.scalar.copy(ms_r[:, sl], st_q[:])
                            nc.vector.tensor_mul(sd_r[:, sl], mu_r[:, sl],
                                                 mu_r[:, sl])
                            nc.vector.tensor_sub(sd_r[:, sl], ms_r[:, sl],
                                                 sd_r[:, sl])
                            nc.scalar.activation(sd_r[:, sl], sd_r[:, sl],
                                                 AF.Sqrt,
                                                 bias=c_eps[0:1, :])
                            nc.vector.reciprocal_approx_fast(rc_r[:, sl],
                                                             sd_r[:, sl])
                            pm = psc.tile([128, 512], F32, tag="pj", bufs=2,
                                          name="pm")
                            nc.tensor.matmul(pm[:], v32(c_on[0:1, :]),
                                             mu_r[:, sl])
                            nc.scalar.copy(v32(mu_b[:, sl]), pm[:])
                            pr = psc.tile([128, 512], F32, tag="pj", bufs=2,
                                          name="pr")
                            nc.tensor.matmul(pr[:], v32(c_on[0:1, :]),
                                             rc_r[:, sl])
                            nc.scalar.copy(v32(r_b[:, sl]), pr[:])
                            for c in range(ND):
                                nc.vector.tensor_sub(xln1[c][:, sl],
                                                     xln1[c][:, sl],
                                                     mu_b[:, sl])
                                nc.vector.tensor_mul(xln1[c][:, sl],
                                                     xln1[c][:, sl],
                                                     r_b[:, sl])
                                nc.vector.tensor_scalar(
                                    out=xln1[c][:, sl],
                                    in0=xln1[c][:, sl],
                                    scalar1=c_g2[:, c:c + 1],
                                    scalar2=c_be2[:, c:c + 1],
                                    op0=ALU.mult, op1=ALU.add)
                                nc.scalar.copy(xb[c][:, sl],
                                               xln1[c][:, sl])
                            for m in range(4 * jj, 4 * (jj + 1)):
                                for c in range(ND):
                                    cs = slice(128 * c, 128 * (c + 1))
                                    pt = psc.tile([128, 128], F32R,
                                                  tag="tr3", bufs=2,
                                                  name="pt3")
                                    nc.tensor.transpose(
                                        pt[:],
                                        xln1[c][:, 128 * m:128 * (m + 1)],
                                        c_id[:])
                                    nc.vector.tensor_add(
                                        xp2[m][:, cs], pt[:], c_b2[:, cs])
                    xln2 = xln1
                    if upto == 5:
                        dump_fm(xln2, ND)
                        raise _Done()

                  with (
                      tc.tile_pool(name="acc", bufs=8) as accp,
                      tc.tile_pool(name="hj", bufs=16) as hjp,
                      tc.tile_pool(name="w1t", bufs=4) as w1p,
                      tc.tile_pool(name="w2t", bufs=16) as w2p,
                      tc.tile_pool(name="ps_d", bufs=1, space="PSUM") as psd,
                  ):
                    acc = [accp.tile([128, D], F32, tag="acc", name="acc")
                           for _ in range(NT)]
                    hjs = [[hjp.tile([128, T], BF16, tag="hj", name="hj")
                            for _ in range(8)] for _ in range(2)]
                    w2s = [[w2p.tile([128, D], BF16, tag="w2", name="w2t")
                            for _ in range(8)] for _ in range(2)]
                    for jg in range(4):
                        hj = hjs[jg % 2]
                        w2_t = w2s[jg % 2]
                        for j8 in range(8):
                            j = 8 * jg + j8
                            w1t = w1p.tile([128, D], BF16, tag="w1")
                            nc.sync.dma_start(
                                w1t[:], w1r[j].rearrange("a b c -> a (b c)"))
                            f1 = psd.tile([128, T], F32, tag="f1", bufs=2,
                                          name="f1")
                            for jj in range(2):
                                sl = slice(512 * jj, 512 * (jj + 1))
                                for c in range(ND):
                                    nc.tensor.matmul(
                                        f1[:, sl],
                                        w1t[:, 128 * c:128 * (c + 1)],
                                        xb[c][:, sl],
                                        start=(c == 0), stop=(c == ND - 1))
                            nc.vector.tensor_scalar(
                                out=hj[j8][:], in0=f1[:],
                                scalar1=c_b1[:, j:j + 1], scalar2=0.0,
                                op0=ALU.add, op1=ALU.max)
                            nc.sync.dma_start(
                                w2_t[j8][:], w2b[128 * j:128 * (j + 1), :])
                        for m in range(NT):
                            for nb in range(2):
                                sl = slice(512 * nb, 512 * (nb + 1))
                                fb = psd.tile([128, 512], F32, tag="fb",
                                              bufs=4, name="fb")
                                for j8 in range(8):
                                    nc.tensor.matmul(
                                        fb[:],
                                        hj[j8][:, 128 * m:128 * (m + 1)],
                                        w2_t[j8][:, sl],
                                        start=(j8 == 0), stop=(j8 == 7))
                                if jg == 0:
                                    nc.vector.tensor_copy(acc[m][:, sl],
                                                          fb[:])
                                else:
                                    nc.vector.tensor_add(
                                        acc[m][:, sl], fb[:],
                                        acc[m][:, sl])
                    for m in range(NT):
                        nc.vector.tensor_add(xp2[m][:], acc[m][:], xp2[m][:])
                        nc.sync.dma_start(
                            out_l[128 * m:128 * (m + 1), :], xp2[m][:])

            if reps > 1:
                with tc.For_i(0, reps, 1):
                    _phases()
            else:
                _phases()
          except _Done:
            pass

    nc.compile()
    return nc


_NC = None


def _get_nc():
    global _NC
    if _NC is None:
        _NC = _build()
    return _NC


def _prep_common(wq, wk, wv, w_proj, b_proj, w1, b1, w2, b2, g1, be1, g2, be2):
    f = np.float32
    bf = ml_dtypes.bfloat16
    wq = np.asarray(wq, f)
    wk = np.asarray(wk, f)
    wv = np.asarray(wv, f)

    def pack_pairs(w):
        w5 = w.reshape(H // 2, 2, ND, 128, E)
        return np.ascontiguousarray(
            w5.transpose(0, 3, 2, 1, 4).reshape(H // 2, 128, ND, 128)
            .astype(bf))

    w1 = np.asarray(w1, f)
    return {
        "wqp": pack_pairs(wq),
        "wkp": pack_pairs(wk),
        "wva": np.ascontiguousarray(
            wv.transpose(1, 0, 2).reshape(D, D).astype(bf)),
        "wpj": np.ascontiguousarray(np.asarray(w_proj, f).astype(bf)),
        "w1r": np.ascontiguousarray(
            w1.reshape(ND, 128, NJ, 128).transpose(2, 1, 0, 3).astype(bf)),
        "w2b": np.ascontiguousarray(np.asarray(w2, f).astype(bf)),
        "g1f": np.ascontiguousarray(np.asarray(g1, f).reshape(ND, 128).T),
        "be1f": np.ascontiguousarray(np.asarray(be1, f).reshape(ND, 128).T),
        "g2f": np.ascontiguousarray(np.asarray(g2, f).reshape(ND, 128).T),
        "be2f": np.ascontiguousarray(np.asarray(be2, f).reshape(ND, 128).T),
        "bpf": np.ascontiguousarray(np.asarray(b_proj, f).reshape(ND, 128).T),
        "b1f": np.ascontiguousarray(np.asarray(b1, f).reshape(NJ, 128).T),
        "b2b": np.ascontiguousarray(np.tile(np.asarray(b2, f), (128, 1))),
        "idn": np.eye(128, dtype=f),
        "onz": np.ones((128, 128), f),
        "ond": np.full((128, 1), 1.0 / D, f),
        "mby": np.where(np.arange(128)[None, :] >= np.arange(128)[:, None],
                        1.0, 0.0).astype(bf),
    }


def kernel(x, wq, wk, wv, w_proj, b_proj, w1, b1, w2, b2, g1, be1, g2, be2,
           **bench):
    nc = _get_nc()
    common = _prep_common(wq, wk, wv, w_proj, b_proj, w1, b1, w2, b2,
                          g1, be1, g2, be2)
    x = np.asarray(x, np.float32)
    in_maps = [dict(common, x_l=np.ascontiguousarray(x[b]))
               for b in range(NCORE)]
    res = bass_utils.run_bass_kernel_spmd(
        nc, in_maps, core_ids=list(range(NCORE)), **bench)
    out = np.stack([res.results[b]["out_l"] for b in range(NCORE)])
    if bench:
        kernel.last_results = res
    return out


if __name__ == "__main__":
    _build()
    print("built ok")
